# revision 1
# baseline (speedup 1.0000x reference)
"""Causal self-attention with RoPE for TRN2, sharded over 8 NeuronCores.

Sharding (Megatron-style tensor parallel on heads):
  - 16 heads -> 2 heads per core; each core also handles both batch rows.
  - Each core computes q/k/v projections for its 2 heads (256 features),
    causal attention for its (b, h) pairs, and a partial output
    projection through its 256 columns of Wo.
  - Host sums the 8 partial outputs (the "all-reduce").

All matmuls run as float32r (full-rate fp32 on the PE array). Everything
is kept in transposed layouts so no attention-side transposes are needed:
  qT/kT/vT: [hd=128, S]   scoresT: [j, q]   attnT: [j, q]   outT: [d, q]
Softmax runs without max-subtraction (scaled scores are O(6), exp is safe);
partition-axis sums use a ones-column matmul; 1/sum is broadcast back to
128 partitions with a K=1 ones-row matmul and folded into the outT evict.
"""
import sys

sys.path.insert(0, "/opt/trn_rl_repo")

import numpy as np
import ml_dtypes

import concourse.bass as bass
import concourse.bacc as bacc
import concourse.mybir as mybir
import concourse.tile as tile
from concourse.bass_utils import run_bass_kernel_spmd

F32 = mybir.dt.float32
F32R = mybir.dt.float32r

B, S, D, H, HD = 2, 2048, 2048, 16, 128
N_CORES = 8
HPC = H // N_CORES          # heads per core = 2
FPC = HPC * HD              # features per core = 256
SCALE = 1.0 / float(np.sqrt(HD))
NKT = D // 128              # 16 contraction tiles
NSC = S // 512              # 4 s-chunks per batch
NJT = S // 128              # 16 key tiles per batch
BS = B * S


def _round_f32r(x):
    x = np.ascontiguousarray(x, dtype=np.float32)
    hi = x.astype(ml_dtypes.bfloat16).astype(np.float32)
    lo = (x - hi).astype(ml_dtypes.bfloat16).astype(np.float32)
    return hi + lo


def build_nc():
    nc = bacc.Bacc(None, target_bir_lowering=False, debug=False)
    Exp = mybir.ActivationFunctionType.Exp

    xT_d = nc.dram_tensor("xT", [D, BS], F32R, kind="ExternalInput")
    wq_d = nc.dram_tensor("wq", [D, FPC], F32R, kind="ExternalInput")
    wk_d = nc.dram_tensor("wk", [D, FPC], F32R, kind="ExternalInput")
    wv_d = nc.dram_tensor("wv", [D, FPC], F32R, kind="ExternalInput")
    wo_d = nc.dram_tensor("wo", [FPC, D], F32R, kind="ExternalInput")
    cos_d = nc.dram_tensor("cos", [128, S], F32R, kind="ExternalInput")
    sin_d = nc.dram_tensor("sin", [128, S], F32R, kind="ExternalInput")
    mask_d = nc.dram_tensor("mask", [128, 896], F32R, kind="ExternalInput")
    rmat_d = nc.dram_tensor("rmat", [128, 128], F32R, kind="ExternalInput")
    ident_d = nc.dram_tensor("ident", [128, 128], F32R, kind="ExternalInput")
    onesc_d = nc.dram_tensor("onesc", [128, 1], F32R, kind="ExternalInput")
    onesr_d = nc.dram_tensor("onesr", [1, 128], F32R, kind="ExternalInput")
    out_d = nc.dram_tensor("outP", [D, BS], F32, kind="ExternalOutput")

    # group kt tiles in fours so each x DMA moves 1 MiB in one descriptor
    xT_r = xT_d[:].rearrange("(n t p) s -> n p t s", p=128, t=4)
    wq_r = wq_d[:].rearrange("(g t p) f -> g p t f", p=128, t=4)
    wk_r = wk_d[:].rearrange("(g t p) f -> g p t f", p=128, t=4)
    wv_r = wv_d[:].rearrange("(g t p) f -> g p t f", p=128, t=4)
    wo_r = wo_d[:].rearrange("(ft p) d -> p ft d", p=128)
    out_r = out_d[:].rearrange("(dt p) s -> dt p s", p=128)

    with tile.TileContext(nc) as tc:
        with (
            nc.allow_low_precision(reason="f32r matmul rounding is intended"),
            tc.tile_pool(name="const", bufs=1) as constp,
            tc.tile_pool(name="xt", bufs=2) as xtp,
            tc.tile_pool(name="qkv", bufs=1) as qkvp,
            tc.tile_pool(name="vh", bufs=2) as vhp,
            tc.tile_pool(name="rope", bufs=2) as ropep,
            tc.tile_pool(name="attn", bufs=5) as attnp,
            tc.tile_pool(name="small", bufs=2) as smallp,
            tc.tile_pool(name="osb", bufs=1) as osbp,
            tc.tile_pool(name="wot", bufs=2) as wotp,
            tc.tile_pool(name="outev", bufs=3) as outevp,
            tc.tile_pool(name="pacc", bufs=6, space="PSUM") as paccp,
            tc.tile_pool(name="pav", bufs=1, space="PSUM") as pavp,
            tc.tile_pool(name="psum1", bufs=1, space="PSUM") as psum1p,
        ):
            # ---- constants ----
            wq_g, wk_g, wv_g = [], [], []
            for g in range(NKT // 4):
                wqt = constp.tile([128, 4, FPC], F32R, name=f"wq_g{g}")
                wkt = constp.tile([128, 4, FPC], F32R, name=f"wk_g{g}")
                wvt = constp.tile([128, 4, FPC], F32R, name=f"wv_g{g}")
                weng = nc.sync if g == 0 else nc.scalar
                weng.dma_start(wqt[:], wq_r[g])
                weng.dma_start(wkt[:], wk_r[g])
                weng.dma_start(wvt[:], wv_r[g])
                wq_g.append(wqt); wk_g.append(wkt); wv_g.append(wvt)
            cos_sb = constp.tile([128, S], F32R)
            sin_sb = constp.tile([128, S], F32R)
            nc.scalar.dma_start(cos_sb[:], cos_d[:])
            nc.scalar.dma_start(sin_sb[:], sin_d[:])
            mask_sb = constp.tile([128, 896], F32R)
            nc.scalar.dma_start(mask_sb[:], mask_d[:])
            rmat_sb = constp.tile([128, 128], F32R)
            ident_sb = constp.tile([128, 128], F32R)
            onesc_sb = constp.tile([128, 1], F32R)
            onesr_sb = constp.tile([1, 128], F32R)
            nc.scalar.dma_start(rmat_sb[:], rmat_d[:])
            nc.scalar.dma_start(ident_sb[:], ident_d[:])
            nc.scalar.dma_start(onesc_sb[:], onesc_d[:])
            nc.scalar.dma_start(onesr_sb[:], onesr_d[:])

            for b in range(B):
                o_sb = osbp.tile([128, HPC, S], F32R, tag="o_sb")
                qTs, kTs, vTs = [], [], []
                for h in range(HPC):
                    qTs.append(qkvp.tile([128, S], F32R, name=f"qT{h}", tag=f"qT{h}"))
                    kTs.append(qkvp.tile([128, S], F32R, name=f"kT{h}", tag=f"kT{h}"))
                    vTs.append(qkvp.tile([128, S], F32R, name=f"vT{h}", tag=f"vT{h}"))
                # ---- projections: both heads share each xT tile ----
                for sc in range(NSC):
                    ss = slice(512 * sc, 512 * sc + 512)
                    acc = [paccp.tile([128, 512], F32, name=f"pa{j}", tag="pacc")
                           for j in range(6)]
                    for g in range(NKT // 4):
                        xt = xtp.tile([128, 4, 512], F32R, tag="xt")
                        eng = nc.sync if g % 2 == 0 else nc.gpsimd
                        eng.dma_start(
                            xt[:], xT_r[g, :, :, b * S + 512 * sc:
                                        b * S + 512 * sc + 512])
                        for i in range(4):
                            kt = 4 * g + i
                            st, sp = kt == 0, kt == NKT - 1
                            for h in range(HPC):
                                fs = slice(128 * h, 128 * h + 128)
                                nc.tensor.matmul(acc[h][:], wq_g[g][:, i, fs],
                                                 xt[:, i, :], start=st, stop=sp)
                                nc.tensor.matmul(acc[2 + h][:], wk_g[g][:, i, fs],
                                                 xt[:, i, :], start=st, stop=sp)
                                nc.tensor.matmul(acc[4 + h][:], wv_g[g][:, i, fs],
                                                 xt[:, i, :], start=st, stop=sp)
                    for h in range(HPC):
                        nc.scalar.copy(qTs[h][:, ss], acc[h][:])
                        nc.scalar.copy(kTs[h][:, ss], acc[2 + h][:])
                        nc.scalar.copy(vTs[h][:, ss], acc[4 + h][:])
                for h in range(HPC):
                    qT, kT, vT = qTs[h], kTs[h], vTs[h]
                    # ---- RoPE in place on qT, kT ----
                    for t_ in (qT, kT):
                        for sc in range(NSC):
                            ss = slice(512 * sc, 512 * sc + 512)
                            ps_rot = paccp.tile([128, 512], F32, tag="pacc")
                            nc.tensor.matmul(ps_rot[:], rmat_sb[:], t_[:, ss],
                                             start=True, stop=True)
                            t2 = ropep.tile([128, 512], F32R, tag="ropetmp")
                            nc.vector.tensor_mul(t2[:], ps_rot[:], sin_sb[:, ss])
                            t1 = ropep.tile([128, 512], F32R, tag="ropetmp")
                            nc.vector.tensor_mul(t1[:], t_[:, ss], cos_sb[:, ss])
                            nc.vector.tensor_add(t_[:, ss], t1[:], t2[:])
                    # ---- transpose vT -> v_h [j, jt, d] ----
                    v_h = vhp.tile([128, NJT, 128], F32R, tag="v_h")
                    for jt in range(NJT):
                        js = slice(128 * jt, 128 * jt + 128)
                        ps_tp = paccp.tile([128, 128], F32R, tag="pacc")
                        nc.tensor.transpose(ps_tp[:], vT[:, js], ident_sb[:])
                        nc.scalar.copy(v_h[:, jt, :], ps_tp[:])
                    # ---- attention ----
                    for qc in range(NSC):
                        qs = slice(512 * qc, 512 * qc + 512)
                        ps_av = pavp.tile([128, 512], F32, tag="pav")
                        ps_sum = psum1p.tile([1, 512], F32, tag="psum1")
                        njt = 4 * qc + 4
                        for jt in range(njt):
                            js = slice(128 * jt, 128 * jt + 128)
                            ps_sc = paccp.tile([128, 512], F32, tag="pacc")
                            nc.tensor.matmul(ps_sc[:], kT[:, js], qT[:, qs],
                                             start=True, stop=True)
                            at = attnp.tile([128, 512], F32R, tag="at")
                            nc.scalar.activation(at[:], ps_sc[:], Exp,
                                                 scale=SCALE)
                            if jt >= 4 * qc:
                                mi = 384 - 128 * (jt - 4 * qc)
                                nc.vector.tensor_mul(
                                    at[:], at[:], mask_sb[:, mi:mi + 512])
                            st, sp = jt == 0, jt == njt - 1
                            nc.tensor.matmul(ps_sum[:], onesc_sb[:], at[:],
                                             start=st, stop=sp)
                            nc.tensor.matmul(ps_av[:], v_h[:, jt, :], at[:],
                                             start=st, stop=sp)
                        sums_sb = smallp.tile([1, 512], F32, tag="sums")
                        nc.scalar.copy(sums_sb[:], ps_sum[:])
                        recip = smallp.tile([1, 512], F32R, tag="recip")
                        nc.vector.reciprocal(recip[:], sums_sb[:])
                        ps_bc = paccp.tile([128, 512], F32, tag="pacc")
                        nc.tensor.matmul(ps_bc[:], onesr_sb[:], recip[:],
                                         start=True, stop=True)
                        recipT = smallp.tile([128, 512], F32, tag="recipT")
                        nc.scalar.copy(recipT[:], ps_bc[:])
                        nc.vector.tensor_mul(o_sb[:, h, qs], ps_av[:],
                                             recipT[:])
                # ---- output projection partial for batch b ----
                for dt in range(D // 128):
                    ds = slice(128 * dt, 128 * dt + 128)
                    wo01 = wotp.tile([128, 2, 128], F32R, tag="wo_t")
                    nc.gpsimd.dma_start(wo01[:], wo_r[:, :, ds])
                    for half in range(2):
                        outt = outevp.tile([128, 1024], F32, tag="outt")
                        for j in range(2):
                            sc = 2 * half + j
                            ss = slice(512 * sc, 512 * sc + 512)
                            ps_o = paccp.tile([128, 512], F32, tag="pacc")
                            nc.tensor.matmul(ps_o[:], wo01[:, 0, :],
                                             o_sb[:, 0, ss],
                                             start=True, stop=False)
                            nc.tensor.matmul(ps_o[:], wo01[:, 1, :],
                                             o_sb[:, 1, ss],
                                             start=False, stop=True)
                            nc.vector.tensor_copy(outt[:, 512 * j:512 * j + 512],
                                                  ps_o[:])
                        oeng = nc.sync if (dt + half) % 2 == 0 else nc.gpsimd
                        oeng.dma_start(
                            out_r[dt, :, b * S + 1024 * half:
                                  b * S + 1024 * half + 1024], outt[:])

    nc.compile()
    return nc


_NC_CACHE = None


def _get_nc():
    global _NC_CACHE
    if _NC_CACHE is None:
        _NC_CACHE = build_nc()
    return _NC_CACHE


def _host_consts():
    inv_freq = 1.0 / (10000.0 ** (np.arange(0, HD, 2, dtype=np.float32) / HD))
    t = np.arange(S, dtype=np.float32)
    freqs = np.outer(t, inv_freq)
    emb = np.concatenate([freqs, freqs], axis=-1)          # [S, hd]
    cosT = _round_f32r(np.cos(emb).T)                       # [hd, S]
    sinT = _round_f32r(np.sin(emb).T)
    # staircase mask: variant i is the slice [:, 384-128i : 384-128i+512]
    r = np.arange(128)[:, None]
    u = np.arange(896)[None, :]
    mask = (u >= r + 384).astype(np.float32)
    rmat = np.zeros((128, 128), np.float32)
    for m in range(64):
        rmat[m + 64, m] = -1.0
        rmat[m, m + 64] = 1.0
    ident = np.eye(128, dtype=np.float32)
    onesc = np.ones((128, 1), np.float32)
    onesr = np.ones((1, 128), np.float32)
    return cosT, sinT, mask, rmat, ident, onesc, onesr


def _make_in_maps(inputs):
    x = np.ascontiguousarray(np.asarray(inputs["x"]), dtype=np.float32)
    Wq = np.asarray(inputs["Wq"], dtype=np.float32)
    Wk = np.asarray(inputs["Wk"], dtype=np.float32)
    Wv = np.asarray(inputs["Wv"], dtype=np.float32)
    Wo = np.asarray(inputs["Wo"], dtype=np.float32)

    xT = _round_f32r(x.reshape(BS, D).T)                    # [D, B*S]
    cosT, sinT, mask, rmat, ident, onesc, onesr = _host_consts()

    in_maps = []
    for cid in range(N_CORES):
        f0 = cid * FPC
        in_maps.append(dict(
            xT=xT,
            wq=_round_f32r(Wq[f0:f0 + FPC, :].T),
            wk=_round_f32r(Wk[f0:f0 + FPC, :].T),
            wv=_round_f32r(Wv[f0:f0 + FPC, :].T),
            wo=_round_f32r(Wo[:, f0:f0 + FPC].T),
            cos=cosT, sin=sinT, mask=mask, rmat=rmat, ident=ident,
            onesc=onesc, onesr=onesr,
        ))
    return in_maps


def kernel(x, Wq, Wk, Wv, Wo):
    in_maps = _make_in_maps(dict(x=x, Wq=Wq, Wk=Wk, Wv=Wv, Wo=Wo))
    nc = _get_nc()
    res = run_bass_kernel_spmd(nc, in_maps, core_ids=list(range(N_CORES)))
    outT = res.results[0]["outP"]
    for cid in range(1, N_CORES):
        outT = outT + res.results[cid]["outP"]
    return np.ascontiguousarray(outT.T).reshape(B, S, D)



# revision 4
# speedup vs baseline: 1.5431x; 1.5431x over previous
"""Causal self-attention with RoPE for TRN2, sharded over 8 NeuronCores.

Sequence-parallel + head-parallel Megatron sharding, optimized for the
axon/PJRT measurement path where per-exec cost is dominated by shipped
input/output bytes and buffer count, not device compute:

  - Inputs per core: ONE bf16 blob [2048, 1536] = x-shard (512 positions,
    transposed) | WqT | WkT | WvT slices (256 features) | Wo slice.
  - True constants (RoPE tables, causal mask, rotate matrix, identity,
    ones) are baked into the NEFF via inline_tensor - loaded at model
    load, zero per-exec cost.
  - On-device AllGather reconstructs full xT [2048, 4096] per core
    (sequence-parallel gather), so x is shipped exactly once total.
  - Each core computes q/k/v projections + RoPE + causal attention for
    its 2 heads (both batches), then a partial output projection through
    its 256 columns of Wo, position-major.
  - On-device ReduceScatter sums the 8 partials; each core ships back
    only its 512-position slice of the output, in bf16.

All matmuls are bf16 with fp32 PSUM accumulation. Softmax runs without
max-subtraction (scaled scores are O(6)); partition-axis sums use a
ones-column matmul; 1/sum is broadcast back via a K=1 ones-row matmul.
"""
import sys

sys.path.insert(0, "/opt/trn_rl_repo")

import numpy as np
import ml_dtypes

import concourse.bass as bass
import concourse.bacc as bacc
import concourse.mybir as mybir
import concourse.tile as tile
from concourse.bass_utils import run_bass_kernel_spmd

F32 = mybir.dt.float32
F32R = mybir.dt.float32r
BF16 = mybir.dt.bfloat16

B, S, D, H, HD = 2, 2048, 2048, 16, 128
N_CORES = 8
HPC = H // N_CORES          # heads per core = 2
FPC = HPC * HD              # features per core = 256
SCALE = 1.0 / float(np.sqrt(HD))
NKT = D // 128              # 16 contraction tiles
NSC = S // 512              # 4 s-chunks per batch
NJT = S // 128              # 16 key tiles per batch
BS = B * S
SPC = BS // N_CORES         # positions per core = 512

# blob column layout: [ x-shard 512 | wqT 256 | wkT 256 | wvT 256 | wo 256 ]
XC, WQC, WKC, WVC, WOC = 0, 512, 768, 1024, 1280
BLOBC = 1536


def _host_consts():
    inv_freq = 1.0 / (10000.0 ** (np.arange(0, HD, 2, dtype=np.float32) / HD))
    t = np.arange(S, dtype=np.float32)
    freqs = np.outer(t, inv_freq)                           # [S, hd/2]
    cos_h = np.cos(freqs).T.astype(ml_dtypes.bfloat16)      # [64, S]
    sin_h = np.sin(freqs).T.astype(ml_dtypes.bfloat16)
    # staircase mask: variant d is the slice [:, 384-128d : 384-128d+512]
    r = np.arange(128)[:, None]
    u = np.arange(896)[None, :]
    mask = (u >= r + 384).astype(ml_dtypes.bfloat16)
    rmat = np.zeros((128, 128), np.float32)
    for m in range(64):
        rmat[m + 64, m] = -1.0
        rmat[m, m + 64] = 1.0
    rmat = rmat.astype(ml_dtypes.bfloat16)
    ident = np.eye(128, dtype=ml_dtypes.bfloat16)
    onesc = np.ones((128, 1), ml_dtypes.bfloat16)
    onesr = np.ones((1, 128), np.float32)
    return cos_h, sin_h, mask, rmat, ident, onesc, onesr


def build_nc():
    nc = bacc.Bacc(None, target_bir_lowering=False, debug=False)
    Exp = mybir.ActivationFunctionType.Exp
    groups = [[i for i in range(N_CORES)]]

    blob_d = nc.dram_tensor("blob", [D, BLOBC], BF16, kind="ExternalInput")
    outb_d = nc.dram_tensor("outb", [SPC, D], BF16, kind="ExternalOutput")

    cos_h, sin_h, mask_np, rmat_np, ident_np, onesc_np, onesr_np = _host_consts()
    cos_d = nc.inline_tensor(cos_h, "cosc")                 # [64, S] bf16
    sin_d = nc.inline_tensor(sin_h, "sinc")
    mask_d = nc.inline_tensor(mask_np, "maskc")             # [128, 896] bf16
    rmat_d = nc.inline_tensor(rmat_np, "rmatc")
    ident_d = nc.inline_tensor(ident_np, "identc")
    onesc_d = nc.inline_tensor(onesc_np, "onescc")
    onesr_d = nc.inline_tensor(onesr_np.astype(np.float32), "onesrc")  # f32r use

    xstage_d = nc.dram_tensor("xstage", [D, SPC], BF16)     # internal
    xag_d = nc.dram_tensor("xag", [N_CORES * D, SPC], BF16)  # gathered x
    pout_d = nc.dram_tensor("pout", [BS, D], F32)           # partial out
    rsout_d = nc.dram_tensor("rsout", [SPC, D], F32)        # reduced slice

    blob_r = blob_d[:].rearrange("(g t p) c -> g p t c", p=128, t=4)  # [4,...]
    xstage_r = xstage_d[:].rearrange("(t p) s -> p t s", p=128)       # 16 t
    xag_r = xag_d[:].rearrange("(n t p) s -> n p t s", p=128, t=4)    # 32 n
    pout_r = pout_d[:].rearrange("(t p) d -> t p d", p=128)           # 32 t
    rsout_r = rsout_d[:].rearrange("(t p) d -> t p d", p=128)         # 4 t
    outb_r = outb_d[:].rearrange("(t p) d -> t p d", p=128)           # 4 t

    with tile.TileContext(nc) as tc:
        with (
            nc.allow_low_precision(reason="bf16 compute is intended"),
            tc.tile_pool(name="const", bufs=1) as constp,
            tc.tile_pool(name="xt", bufs=2) as xtp,
            tc.tile_pool(name="qkv", bufs=1) as qkvp,
            tc.tile_pool(name="vh", bufs=2) as vhp,
            tc.tile_pool(name="rope", bufs=2) as ropep,
            tc.tile_pool(name="attn", bufs=5) as attnp,
            tc.tile_pool(name="small", bufs=2) as smallp,
            tc.tile_pool(name="osb", bufs=1) as osbp,
            tc.tile_pool(name="outev", bufs=3) as outevp,
            tc.tile_pool(name="pacc", bufs=6, space="PSUM") as paccp,
            tc.tile_pool(name="pav", bufs=1, space="PSUM") as pavp,
            tc.tile_pool(name="psum1", bufs=1, space="PSUM") as psum1p,
        ):
            # ---- stage x-shard to internal dram, then AllGather ----
            xst = constp.tile([128, 16, SPC], BF16, name="xst")
            nc.sync.dma_start(
                xst[:], blob_d[:].rearrange("(t p) c -> p t c", p=128)
                [:, :, XC:XC + SPC])
            nc.sync.dma_start(xstage_r, xst[:])
            nc.gpsimd.collective_compute(
                "AllGather", mybir.AluOpType.bypass, replica_groups=groups,
                ins=[xstage_d[:].opt()], outs=[xag_d[:].opt()])

            # ---- constants to SBUF ----
            wall_g = []
            for g in range(4):
                wt = constp.tile([128, 4, 1024], BF16, name=f"w_g{g}")
                nc.scalar.dma_start(wt[:], blob_r[g][:, :, WQC:BLOBC])
                wall_g.append(wt)
            cos_sb = constp.tile([128, S], BF16)
            sin_sb = constp.tile([128, S], BF16)
            nc.scalar.dma_start(cos_sb[0:64, :], cos_d[:])
            nc.scalar.dma_start(cos_sb[64:128, :], cos_d[:])
            nc.scalar.dma_start(sin_sb[0:64, :], sin_d[:])
            nc.scalar.dma_start(sin_sb[64:128, :], sin_d[:])
            mask_sb = constp.tile([128, 896], BF16)
            nc.scalar.dma_start(mask_sb[:], mask_d[:])
            rmat_sb = constp.tile([128, 128], BF16)
            ident_sb = constp.tile([128, 128], BF16)
            onesc_sb = constp.tile([128, 1], BF16)
            onesr_sb = constp.tile([1, 128], F32R)
            nc.scalar.dma_start(rmat_sb[:], rmat_d[:])
            nc.scalar.dma_start(ident_sb[:], ident_d[:])
            nc.scalar.dma_start(onesc_sb[:], onesc_d[:])
            nc.gpsimd.dma_start(onesr_sb[:], onesr_d[:])

            # ---- transpose Wo slice [d, f] -> wo_sb [f, 2, d] ----
            wo_sb = constp.tile([128, 2, D], BF16, name="wo_sb")
            for g in range(4):
                for i in range(4):
                    for f2 in range(2):
                        ps_tp = paccp.tile([128, 128], BF16, tag="pacc")
                        nc.tensor.transpose(
                            ps_tp[:],
                            wall_g[g][:, i, 768 + 128 * f2:768 + 128 * (f2 + 1)],
                            ident_sb[:])
                        nc.scalar.copy(wo_sb[:, f2, 512 * g + 128 * i:
                                             512 * g + 128 * i + 128], ps_tp[:])

            for b in range(B):
                o_sb = osbp.tile([128, HPC, S], BF16, tag="o_sb")
                qTs, kTs, vTs = [], [], []
                for h in range(HPC):
                    qTs.append(qkvp.tile([128, S], BF16, name=f"qT{h}", tag=f"qT{h}"))
                    kTs.append(qkvp.tile([128, S], BF16, name=f"kT{h}", tag=f"kT{h}"))
                    vTs.append(qkvp.tile([128, S], BF16, name=f"vT{h}", tag=f"vT{h}"))
                # ---- projections: both heads share each xag tile ----
                for sc in range(NSC):
                    ss = slice(512 * sc, 512 * sc + 512)
                    acc = [paccp.tile([128, 512], F32, name=f"pa{j}", tag="pacc")
                           for j in range(6)]
                    for g in range(4):
                        xt = xtp.tile([128, 4, 512], BF16, tag="xt")
                        eng = nc.sync if g % 2 == 0 else nc.gpsimd
                        eng.dma_start(xt[:], xag_r[(b * NSC + sc) * 4 + g])
                        for i in range(4):
                            kt = 4 * g + i
                            st, sp = kt == 0, kt == NKT - 1
                            for h in range(HPC):
                                fq = slice(128 * h, 128 * h + 128)
                                fk = slice(256 + 128 * h, 256 + 128 * h + 128)
                                fv = slice(512 + 128 * h, 512 + 128 * h + 128)
                                nc.tensor.matmul(acc[h][:], wall_g[g][:, i, fq],
                                                 xt[:, i, :], start=st, stop=sp)
                                nc.tensor.matmul(acc[2 + h][:], wall_g[g][:, i, fk],
                                                 xt[:, i, :], start=st, stop=sp)
                                nc.tensor.matmul(acc[4 + h][:], wall_g[g][:, i, fv],
                                                 xt[:, i, :], start=st, stop=sp)
                    for h in range(HPC):
                        nc.scalar.copy(qTs[h][:, ss], acc[h][:])
                        nc.scalar.copy(kTs[h][:, ss], acc[2 + h][:])
                        nc.scalar.copy(vTs[h][:, ss], acc[4 + h][:])
                for h in range(HPC):
                    qT, kT, vT = qTs[h], kTs[h], vTs[h]
                    # ---- RoPE in place on qT, kT ----
                    for t_ in (qT, kT):
                        for sc in range(NSC):
                            ss = slice(512 * sc, 512 * sc + 512)
                            ps_rot = paccp.tile([128, 512], F32, tag="pacc")
                            nc.tensor.matmul(ps_rot[:], rmat_sb[:], t_[:, ss],
                                             start=True, stop=True)
                            t2 = ropep.tile([128, 512], BF16, tag="ropetmp")
                            nc.vector.tensor_mul(t2[:], ps_rot[:], sin_sb[:, ss])
                            t1 = ropep.tile([128, 512], BF16, tag="ropetmp")
                            nc.vector.tensor_mul(t1[:], t_[:, ss], cos_sb[:, ss])
                            nc.vector.tensor_add(t_[:, ss], t1[:], t2[:])
                    # ---- transpose vT -> v_h [j, jt, d] ----
                    v_h = vhp.tile([128, NJT, 128], BF16, tag="v_h")
                    for jt in range(NJT):
                        js = slice(128 * jt, 128 * jt + 128)
                        ps_tp = paccp.tile([128, 128], BF16, tag="pacc")
                        nc.tensor.transpose(ps_tp[:], vT[:, js], ident_sb[:])
                        nc.scalar.copy(v_h[:, jt, :], ps_tp[:])
                    # ---- attention ----
                    for qc in range(NSC):
                        qs = slice(512 * qc, 512 * qc + 512)
                        ps_av = pavp.tile([128, 512], F32, tag="pav")
                        ps_sum = psum1p.tile([1, 512], F32, tag="psum1")
                        njt = 4 * qc + 4
                        for jt in range(njt):
                            js = slice(128 * jt, 128 * jt + 128)
                            ps_sc = paccp.tile([128, 512], F32, tag="pacc")
                            nc.tensor.matmul(ps_sc[:], kT[:, js], qT[:, qs],
                                             start=True, stop=True)
                            at = attnp.tile([128, 512], BF16, tag="at")
                            nc.scalar.activation(at[:], ps_sc[:], Exp,
                                                 scale=SCALE)
                            if jt >= 4 * qc:
                                mi = 384 - 128 * (jt - 4 * qc)
                                nc.vector.tensor_mul(
                                    at[:], at[:], mask_sb[:, mi:mi + 512])
                            st, sp = jt == 0, jt == njt - 1
                            nc.tensor.matmul(ps_sum[:], onesc_sb[:], at[:],
                                             start=st, stop=sp)
                            nc.tensor.matmul(ps_av[:], v_h[:, jt, :], at[:],
                                             start=st, stop=sp)
                        sums_sb = smallp.tile([1, 512], F32, tag="sums")
                        nc.scalar.copy(sums_sb[:], ps_sum[:])
                        recip = smallp.tile([1, 512], F32R, tag="recip")
                        nc.vector.reciprocal(recip[:], sums_sb[:])
                        ps_bc = paccp.tile([128, 512], F32, tag="pacc")
                        nc.tensor.matmul(ps_bc[:], onesr_sb[:], recip[:],
                                         start=True, stop=True)
                        recipT = smallp.tile([128, 512], F32, tag="recipT")
                        nc.scalar.copy(recipT[:], ps_bc[:])
                        nc.vector.tensor_mul(o_sb[:, h, qs], ps_av[:],
                                             recipT[:])
                # ---- partial output projection, position-major ----
                for st in range(S // 128):
                    sl = slice(128 * st, 128 * st + 128)
                    for dc in range(4):
                        dsl = slice(512 * dc, 512 * dc + 512)
                        ps_o = paccp.tile([128, 512], F32, tag="pacc")
                        nc.tensor.matmul(ps_o[:], o_sb[:, 0, sl],
                                         wo_sb[:, 0, dsl],
                                         start=True, stop=False)
                        nc.tensor.matmul(ps_o[:], o_sb[:, 1, sl],
                                         wo_sb[:, 1, dsl],
                                         start=False, stop=True)
                        outt = outevp.tile([128, 512], F32, tag="outt")
                        nc.vector.tensor_copy(outt[:], ps_o[:])
                        oeng = nc.sync if (st + dc) % 2 == 0 else nc.gpsimd
                        oeng.dma_start(pout_r[b * (S // 128) + st][:, dsl],
                                       outt[:])

            # ---- ReduceScatter partials; ship back bf16 slice ----
            nc.gpsimd.collective_compute(
                "ReduceScatter", mybir.AluOpType.add, replica_groups=groups,
                ins=[pout_d[:].opt()], outs=[rsout_d[:].opt()])
            for t_ in range(4):
                rt = outevp.tile([128, D], F32, tag="rsf32")
                nc.sync.dma_start(rt[:], rsout_r[t_])
                rb = outevp.tile([128, D], BF16, tag="rsbf")
                nc.vector.tensor_copy(rb[:], rt[:])
                nc.sync.dma_start(outb_r[t_], rb[:])

    nc.compile()
    return nc


_NC_CACHE = None


def _get_nc():
    global _NC_CACHE
    if _NC_CACHE is None:
        _NC_CACHE = build_nc()
    return _NC_CACHE


def _make_in_maps(inputs):
    x = np.ascontiguousarray(np.asarray(inputs["x"]), dtype=np.float32)
    Wq = np.asarray(inputs["Wq"], dtype=np.float32)
    Wk = np.asarray(inputs["Wk"], dtype=np.float32)
    Wv = np.asarray(inputs["Wv"], dtype=np.float32)
    Wo = np.asarray(inputs["Wo"], dtype=np.float32)

    xT = x.reshape(BS, D).T                                  # [D, B*S]
    in_maps = []
    for cid in range(N_CORES):
        f0 = cid * FPC
        blob = np.empty((D, BLOBC), dtype=ml_dtypes.bfloat16)
        blob[:, XC:XC + SPC] = xT[:, cid * SPC:(cid + 1) * SPC]
        blob[:, WQC:WQC + FPC] = Wq[f0:f0 + FPC, :].T
        blob[:, WKC:WKC + FPC] = Wk[f0:f0 + FPC, :].T
        blob[:, WVC:WVC + FPC] = Wv[f0:f0 + FPC, :].T
        blob[:, WOC:WOC + FPC] = Wo[:, f0:f0 + FPC]
        in_maps.append(dict(blob=blob))
    return in_maps


def kernel(x, Wq, Wk, Wv, Wo):
    in_maps = _make_in_maps(dict(x=x, Wq=Wq, Wk=Wk, Wv=Wv, Wo=Wo))
    nc = _get_nc()
    res = run_bass_kernel_spmd(nc, in_maps, core_ids=list(range(N_CORES)))
    out = np.concatenate(
        [np.asarray(res.results[c]["outb"], dtype=np.float32)
         for c in range(N_CORES)], axis=0)                   # [B*S, D]
    return np.ascontiguousarray(out).reshape(B, S, D)


# revision 7
# speedup vs baseline: 1.7084x; 1.1071x over previous
"""Causal self-attention with RoPE for TRN2, sharded over 8 NeuronCores.

Sequence-parallel + head-parallel Megatron sharding, optimized for the
axon/PJRT measurement path where per-exec cost is dominated by shipped
input/output bytes and buffer count, not device compute:

  - Inputs per core: ONE bf16 blob [2048, 1536] = x-shard (512 positions,
    transposed) | WqT | WkT | WvT slices (256 features) | Wo slice.
  - True constants (RoPE tables, causal mask, rotate matrix, identity,
    ones) are baked into the NEFF via inline_tensor - loaded at model
    load, zero per-exec cost.
  - On-device AllGather reconstructs full xT [2048, 4096] per core
    (sequence-parallel gather), so x is shipped exactly once total.
  - Each core computes q/k/v projections + RoPE + causal attention for
    its 2 heads (both batches), then a partial output projection through
    its 256 columns of Wo, position-major.
  - On-device ReduceScatter sums the 8 partials; each core ships back
    only its 512-position slice of the output, in bf16.

All matmuls are bf16 with fp32 PSUM accumulation. Softmax runs without
max-subtraction (scaled scores are O(6)); partition-axis sums use a
ones-column matmul; 1/sum is broadcast back via a K=1 ones-row matmul.
"""
import sys

sys.path.insert(0, "/opt/trn_rl_repo")

import numpy as np
import ml_dtypes

import concourse.bass as bass
import concourse.bacc as bacc
import concourse.mybir as mybir
import concourse.tile as tile
from concourse.bass_utils import run_bass_kernel_spmd

F32 = mybir.dt.float32
F32R = mybir.dt.float32r
BF16 = mybir.dt.bfloat16

B, S, D, H, HD = 2, 2048, 2048, 16, 128
N_CORES = 8
HPC = H // N_CORES          # heads per core = 2
FPC = HPC * HD              # features per core = 256
SCALE = 1.0 / float(np.sqrt(HD))
NKT = D // 128              # 16 contraction tiles
NSC = S // 512              # 4 s-chunks per batch
NJT = S // 128              # 16 key tiles per batch
BS = B * S
SPC = BS // N_CORES         # positions per core = 512

# blob column layout: [ x-shard 512 | wqT 256 | wkT 256 | wvT 256 | wo 256 ]
XC, WQC, WKC, WVC, WOC = 0, 512, 768, 1024, 1280
BLOBC = 1536


def _host_consts():
    inv_freq = 1.0 / (10000.0 ** (np.arange(0, HD, 2, dtype=np.float32) / HD))
    t = np.arange(S, dtype=np.float32)
    freqs = np.outer(t, inv_freq)                           # [S, hd/2]
    cos_h = np.cos(freqs).T.astype(ml_dtypes.bfloat16)      # [64, S]
    sin_h = np.sin(freqs).T.astype(ml_dtypes.bfloat16)
    # staircase mask: variant d is the slice [:, 384-128d : 384-128d+512]
    r = np.arange(128)[:, None]
    u = np.arange(896)[None, :]
    mask = (u >= r + 384).astype(ml_dtypes.bfloat16)
    rmat = np.zeros((128, 128), np.float32)
    for m in range(64):
        rmat[m + 64, m] = -1.0
        rmat[m, m + 64] = 1.0
    rmat = rmat.astype(ml_dtypes.bfloat16)
    ident = np.eye(128, dtype=ml_dtypes.bfloat16)
    onesc = np.ones((128, 1), ml_dtypes.bfloat16)
    onesr = np.ones((1, 128), np.float32)
    return cos_h, sin_h, mask, rmat, ident, onesc, onesr


def build_nc():
    nc = bacc.Bacc(None, target_bir_lowering=False, debug=False)
    Exp = mybir.ActivationFunctionType.Exp
    groups = [[i for i in range(N_CORES)]]

    blob_d = nc.dram_tensor("blob", [D, BLOBC], BF16, kind="ExternalInput")
    outb_d = nc.dram_tensor("outb", [SPC, D], BF16, kind="ExternalOutput")

    cos_h, sin_h, mask_np, rmat_np, ident_np, onesc_np, onesr_np = _host_consts()
    cos_d = nc.inline_tensor(cos_h, "cosc")                 # [64, S] bf16
    sin_d = nc.inline_tensor(sin_h, "sinc")
    mask_d = nc.inline_tensor(mask_np, "maskc")             # [128, 896] bf16
    rmat_d = nc.inline_tensor(rmat_np, "rmatc")
    ident_d = nc.inline_tensor(ident_np, "identc")
    onesc_d = nc.inline_tensor(onesc_np, "onescc")
    onesr_d = nc.inline_tensor(onesr_np.astype(np.float32), "onesrc")  # f32r use

    xstage_d = nc.dram_tensor("xstage", [D, SPC], BF16)     # internal
    xag_d = nc.dram_tensor("xag", [N_CORES * D, SPC], BF16,
                           addr_space="Shared")             # gathered x
    pout_d = nc.dram_tensor("pout", [BS, D], BF16)          # partial out
    rsout_d = nc.dram_tensor("rsout", [SPC, D], BF16)       # reduced slice

    blob_r = blob_d[:].rearrange("(g t p) c -> g p t c", p=128, t=4)  # [4,...]
    xstage_r = xstage_d[:].rearrange("(t p) s -> p t s", p=128)       # 16 t
    xag_r = xag_d[:].rearrange("(n t p) s -> n p t s", p=128, t=4)    # 32 n
    pout_r = pout_d[:].rearrange("(t p) d -> t p d", p=128)           # 32 t
    rsout_r = rsout_d[:].rearrange("(t p) d -> t p d", p=128)         # 4 t
    outb_r = outb_d[:].rearrange("(t p) d -> t p d", p=128)           # 4 t

    with tile.TileContext(nc) as tc:
        with (
            nc.allow_low_precision(reason="bf16 compute is intended"),
            tc.tile_pool(name="const", bufs=1) as constp,
            tc.tile_pool(name="xt", bufs=2) as xtp,
            tc.tile_pool(name="qkv", bufs=1) as qkvp,
            tc.tile_pool(name="vh", bufs=2) as vhp,
            tc.tile_pool(name="rope", bufs=2) as ropep,
            tc.tile_pool(name="attn", bufs=5) as attnp,
            tc.tile_pool(name="small", bufs=2) as smallp,
            tc.tile_pool(name="osb", bufs=1) as osbp,
            tc.tile_pool(name="outev", bufs=3) as outevp,
            tc.tile_pool(name="pacc", bufs=6, space="PSUM") as paccp,
            tc.tile_pool(name="pav", bufs=1, space="PSUM") as pavp,
            tc.tile_pool(name="psum1", bufs=1, space="PSUM") as psum1p,
        ):
            # ---- stage x-shard to internal dram, then AllGather ----
            xst = constp.tile([128, 16, SPC], BF16, name="xst")
            nc.sync.dma_start(
                xst[:], blob_d[:].rearrange("(t p) c -> p t c", p=128)
                [:, :, XC:XC + SPC])
            nc.sync.dma_start(xstage_r, xst[:])
            nc.gpsimd.collective_compute(
                "AllGather", mybir.AluOpType.bypass, replica_groups=groups,
                ins=[xstage_d[:].opt()], outs=[xag_d[:].opt()])

            # ---- constants to SBUF ----
            wall_g = []
            for g in range(4):
                wt = constp.tile([128, 4, 1024], BF16, name=f"w_g{g}")
                nc.scalar.dma_start(wt[:], blob_r[g][:, :, WQC:BLOBC])
                wall_g.append(wt)
            cos_sb = constp.tile([128, S], BF16)
            sin_sb = constp.tile([128, S], BF16)
            nc.scalar.dma_start(cos_sb[0:64, :], cos_d[:])
            nc.scalar.dma_start(cos_sb[64:128, :], cos_d[:])
            nc.scalar.dma_start(sin_sb[0:64, :], sin_d[:])
            nc.scalar.dma_start(sin_sb[64:128, :], sin_d[:])
            mask_sb = constp.tile([128, 896], BF16)
            nc.scalar.dma_start(mask_sb[:], mask_d[:])
            rmat_sb = constp.tile([128, 128], BF16)
            ident_sb = constp.tile([128, 128], BF16)
            onesc_sb = constp.tile([128, 1], BF16)
            onesr_sb = constp.tile([1, 128], F32R)
            nc.scalar.dma_start(rmat_sb[:], rmat_d[:])
            nc.scalar.dma_start(ident_sb[:], ident_d[:])
            nc.scalar.dma_start(onesc_sb[:], onesc_d[:])
            nc.gpsimd.dma_start(onesr_sb[:], onesr_d[:])

            # ---- transpose Wo slice [d, f] -> wo_sb [f, 2, d] ----
            wo_sb = constp.tile([128, 2, D], BF16, name="wo_sb")
            for g in range(4):
                for i in range(4):
                    for f2 in range(2):
                        ps_tp = paccp.tile([128, 128], BF16, tag="pacc")
                        nc.tensor.transpose(
                            ps_tp[:],
                            wall_g[g][:, i, 768 + 128 * f2:768 + 128 * (f2 + 1)],
                            ident_sb[:])
                        nc.scalar.copy(wo_sb[:, f2, 512 * g + 128 * i:
                                             512 * g + 128 * i + 128], ps_tp[:])

            for b in range(B):
                o_sb = osbp.tile([128, HPC, S], BF16, tag="o_sb")
                qTs, kTs, vTs = [], [], []
                for h in range(HPC):
                    qTs.append(qkvp.tile([128, S], BF16, name=f"qT{h}", tag=f"qT{h}"))
                    kTs.append(qkvp.tile([128, S], BF16, name=f"kT{h}", tag=f"kT{h}"))
                    vTs.append(qkvp.tile([128, S], BF16, name=f"vT{h}", tag=f"vT{h}"))
                # ---- projections: both heads share each xag tile ----
                for sc in range(NSC):
                    ss = slice(512 * sc, 512 * sc + 512)
                    acc = [paccp.tile([128, 512], F32, name=f"pa{j}", tag="pacc")
                           for j in range(6)]
                    for g in range(4):
                        xt = xtp.tile([128, 4, 512], BF16, tag="xt")
                        eng = nc.sync if g % 2 == 0 else nc.gpsimd
                        eng.dma_start(xt[:], xag_r[(b * NSC + sc) * 4 + g])
                        for i in range(4):
                            kt = 4 * g + i
                            st, sp = kt == 0, kt == NKT - 1
                            for h in range(HPC):
                                fq = slice(128 * h, 128 * h + 128)
                                fk = slice(256 + 128 * h, 256 + 128 * h + 128)
                                fv = slice(512 + 128 * h, 512 + 128 * h + 128)
                                nc.tensor.matmul(acc[h][:], wall_g[g][:, i, fq],
                                                 xt[:, i, :], start=st, stop=sp)
                                nc.tensor.matmul(acc[2 + h][:], wall_g[g][:, i, fk],
                                                 xt[:, i, :], start=st, stop=sp)
                                nc.tensor.matmul(acc[4 + h][:], wall_g[g][:, i, fv],
                                                 xt[:, i, :], start=st, stop=sp)
                    for h in range(HPC):
                        nc.scalar.copy(qTs[h][:, ss], acc[h][:])
                        nc.scalar.copy(kTs[h][:, ss], acc[2 + h][:])
                        nc.scalar.copy(vTs[h][:, ss], acc[4 + h][:])
                for h in range(HPC):
                    qT, kT, vT = qTs[h], kTs[h], vTs[h]
                    # ---- RoPE in place on qT, kT ----
                    for t_ in (qT, kT):
                        for sc in range(NSC):
                            ss = slice(512 * sc, 512 * sc + 512)
                            ps_rot = paccp.tile([128, 512], F32, tag="pacc")
                            nc.tensor.matmul(ps_rot[:], rmat_sb[:], t_[:, ss],
                                             start=True, stop=True)
                            t2 = ropep.tile([128, 512], BF16, tag="ropetmp")
                            nc.vector.tensor_mul(t2[:], ps_rot[:], sin_sb[:, ss])
                            t1 = ropep.tile([128, 512], BF16, tag="ropetmp")
                            nc.vector.tensor_mul(t1[:], t_[:, ss], cos_sb[:, ss])
                            nc.vector.tensor_add(t_[:, ss], t1[:], t2[:])
                    # ---- transpose vT -> v_h [j, jt, d] ----
                    v_h = vhp.tile([128, NJT, 128], BF16, tag="v_h")
                    for jt in range(NJT):
                        js = slice(128 * jt, 128 * jt + 128)
                        ps_tp = paccp.tile([128, 128], BF16, tag="pacc")
                        nc.tensor.transpose(ps_tp[:], vT[:, js], ident_sb[:])
                        nc.scalar.copy(v_h[:, jt, :], ps_tp[:])
                    # ---- attention ----
                    for qc in range(NSC):
                        qs = slice(512 * qc, 512 * qc + 512)
                        ps_av = pavp.tile([128, 512], F32, tag="pav")
                        ps_sum = psum1p.tile([1, 512], F32, tag="psum1")
                        njt = 4 * qc + 4
                        for jt in range(njt):
                            js = slice(128 * jt, 128 * jt + 128)
                            ps_sc = paccp.tile([128, 512], F32, tag="pacc")
                            nc.tensor.matmul(ps_sc[:], kT[:, js], qT[:, qs],
                                             start=True, stop=True)
                            at = attnp.tile([128, 512], BF16, tag="at")
                            nc.scalar.activation(at[:], ps_sc[:], Exp,
                                                 scale=SCALE)
                            if jt >= 4 * qc:
                                mi = 384 - 128 * (jt - 4 * qc)
                                nc.vector.tensor_mul(
                                    at[:], at[:], mask_sb[:, mi:mi + 512])
                            st, sp = jt == 0, jt == njt - 1
                            nc.tensor.matmul(ps_sum[:], onesc_sb[:], at[:],
                                             start=st, stop=sp)
                            nc.tensor.matmul(ps_av[:], v_h[:, jt, :], at[:],
                                             start=st, stop=sp)
                        sums_sb = smallp.tile([1, 512], F32, tag="sums")
                        nc.scalar.copy(sums_sb[:], ps_sum[:])
                        recip = smallp.tile([1, 512], F32R, tag="recip")
                        nc.vector.reciprocal(recip[:], sums_sb[:])
                        ps_bc = paccp.tile([128, 512], F32, tag="pacc")
                        nc.tensor.matmul(ps_bc[:], onesr_sb[:], recip[:],
                                         start=True, stop=True)
                        recipT = smallp.tile([128, 512], F32, tag="recipT")
                        nc.scalar.copy(recipT[:], ps_bc[:])
                        nc.vector.tensor_mul(o_sb[:, h, qs], ps_av[:],
                                             recipT[:])
                # ---- partial output projection, position-major ----
                for st in range(S // 128):
                    sl = slice(128 * st, 128 * st + 128)
                    for dc in range(4):
                        dsl = slice(512 * dc, 512 * dc + 512)
                        ps_o = paccp.tile([128, 512], F32, tag="pacc")
                        nc.tensor.matmul(ps_o[:], o_sb[:, 0, sl],
                                         wo_sb[:, 0, dsl],
                                         start=True, stop=False)
                        nc.tensor.matmul(ps_o[:], o_sb[:, 1, sl],
                                         wo_sb[:, 1, dsl],
                                         start=False, stop=True)
                        outt = outevp.tile([128, 512], BF16, tag="outt")
                        nc.vector.tensor_copy(outt[:], ps_o[:])
                        oeng = nc.sync if (st + dc) % 2 == 0 else nc.gpsimd
                        oeng.dma_start(pout_r[b * (S // 128) + st][:, dsl],
                                       outt[:])

            # ---- ReduceScatter partials; ship back bf16 slice ----
            nc.gpsimd.collective_compute(
                "ReduceScatter", mybir.AluOpType.add, replica_groups=groups,
                ins=[pout_d[:].opt()], outs=[rsout_d[:].opt()])
            for t_ in range(4):
                rt = outevp.tile([128, D], BF16, tag="rsbf")
                nc.sync.dma_start(rt[:], rsout_r[t_])
                nc.sync.dma_start(outb_r[t_], rt[:])

    nc.compile()
    return nc


_NC_CACHE = None


def _get_nc():
    global _NC_CACHE
    if _NC_CACHE is None:
        _NC_CACHE = build_nc()
    return _NC_CACHE


def _make_in_maps(inputs):
    x = np.ascontiguousarray(np.asarray(inputs["x"]), dtype=np.float32)
    Wq = np.asarray(inputs["Wq"], dtype=np.float32)
    Wk = np.asarray(inputs["Wk"], dtype=np.float32)
    Wv = np.asarray(inputs["Wv"], dtype=np.float32)
    Wo = np.asarray(inputs["Wo"], dtype=np.float32)

    xT = x.reshape(BS, D).T                                  # [D, B*S]
    in_maps = []
    for cid in range(N_CORES):
        f0 = cid * FPC
        blob = np.empty((D, BLOBC), dtype=ml_dtypes.bfloat16)
        blob[:, XC:XC + SPC] = xT[:, cid * SPC:(cid + 1) * SPC]
        blob[:, WQC:WQC + FPC] = Wq[f0:f0 + FPC, :].T
        blob[:, WKC:WKC + FPC] = Wk[f0:f0 + FPC, :].T
        blob[:, WVC:WVC + FPC] = Wv[f0:f0 + FPC, :].T
        blob[:, WOC:WOC + FPC] = Wo[:, f0:f0 + FPC]
        in_maps.append(dict(blob=blob))
    return in_maps


def kernel(x, Wq, Wk, Wv, Wo):
    in_maps = _make_in_maps(dict(x=x, Wq=Wq, Wk=Wk, Wv=Wv, Wo=Wo))
    nc = _get_nc()
    res = run_bass_kernel_spmd(nc, in_maps, core_ids=list(range(N_CORES)))
    out = np.concatenate(
        [np.asarray(res.results[c]["outb"], dtype=np.float32)
         for c in range(N_CORES)], axis=0)                   # [B*S, D]
    return np.ascontiguousarray(out).reshape(B, S, D)


# revision 11
# speedup vs baseline: 1.8014x; 1.0545x over previous
"""Causal self-attention with RoPE for TRN2, sharded over 8 NeuronCores.

Sequence-parallel + head-parallel Megatron sharding, optimized for the
axon/PJRT measurement path where per-exec cost is dominated by shipped
input/output bytes and buffer count, not device compute:

  - Inputs per core: ONE bf16 blob [2048, 1536] = x-shard (512 positions,
    transposed) | WqT | WkT | WvT slices (256 features) | Wo slice.
  - True constants (RoPE tables, causal mask, rotate matrix, identity,
    ones) are baked into the NEFF via inline_tensor - loaded at model
    load, zero per-exec cost.
  - On-device AllGather reconstructs full xT [2048, 4096] per core
    (sequence-parallel gather), so x is shipped exactly once total.
  - Each core computes q/k/v projections + RoPE + causal attention for
    its 2 heads (both batches), then a partial output projection through
    its 256 columns of Wo, position-major.
  - On-device ReduceScatter sums the 8 partials; each core ships back
    only its 512-position slice of the output, in bf16.

All matmuls are bf16 with fp32 PSUM accumulation. Softmax runs without
max-subtraction (scaled scores are O(6)); partition-axis sums use a
ones-column matmul; 1/sum is broadcast back via a K=1 ones-row matmul.
"""
import sys

sys.path.insert(0, "/opt/trn_rl_repo")

import numpy as np
import ml_dtypes

import concourse.bass as bass
import concourse.bacc as bacc
import concourse.mybir as mybir
import concourse.tile as tile
from concourse.bass_utils import run_bass_kernel_spmd

F32 = mybir.dt.float32
F32R = mybir.dt.float32r
BF16 = mybir.dt.bfloat16

B, S, D, H, HD = 2, 2048, 2048, 16, 128
N_CORES = 8
HPC = H // N_CORES          # heads per core = 2
FPC = HPC * HD              # features per core = 256
SCALE = 1.0 / float(np.sqrt(HD))
NKT = D // 128              # 16 contraction tiles
NSC = S // 512              # 4 s-chunks per batch
NJT = S // 128              # 16 key tiles per batch
BS = B * S
SPC = BS // N_CORES         # positions per core = 512

# blob column layout: [ x-shard 512 | wqT 256 | wkT 256 | wvT 256 | wo 256 ]
XC, WQC, WKC, WVC, WOC = 0, 512, 768, 1024, 1280
BLOBC = 1536


def _host_consts():
    inv_freq = 1.0 / (10000.0 ** (np.arange(0, HD, 2, dtype=np.float32) / HD))
    t = np.arange(S, dtype=np.float32)
    freqs = np.outer(t, inv_freq)                           # [S, hd/2]
    cos_h = np.cos(freqs).T.astype(ml_dtypes.bfloat16)      # [64, S]
    sin_h = np.sin(freqs).T.astype(ml_dtypes.bfloat16)
    # staircase mask: variant d is the slice [:, 384-128d : 384-128d+512]
    r = np.arange(128)[:, None]
    u = np.arange(896)[None, :]
    mask = (u >= r + 384).astype(ml_dtypes.bfloat16)
    rmat = np.zeros((128, 128), np.float32)
    for m in range(64):
        rmat[m + 64, m] = -1.0
        rmat[m, m + 64] = 1.0
    rmat = rmat.astype(ml_dtypes.bfloat16)
    ident = np.eye(128, dtype=ml_dtypes.bfloat16)
    onesc = np.ones((128, 1), ml_dtypes.bfloat16)
    onesr = np.ones((1, 128), np.float32)
    return cos_h, sin_h, mask, rmat, ident, onesc, onesr


def build_nc():
    nc = bacc.Bacc(None, target_bir_lowering=False, debug=False)
    Exp = mybir.ActivationFunctionType.Exp
    groups = [[i for i in range(N_CORES)]]

    blob_d = nc.dram_tensor("blob", [D, BLOBC], BF16, kind="ExternalInput")
    outb_d = nc.dram_tensor("outb", [SPC, D], BF16, kind="ExternalOutput")

    cos_h, sin_h, mask_np, rmat_np, ident_np, onesc_np, onesr_np = _host_consts()
    cos_d = nc.inline_tensor(cos_h, "cosc")                 # [64, S] bf16
    sin_d = nc.inline_tensor(sin_h, "sinc")
    mask_d = nc.inline_tensor(mask_np, "maskc")             # [128, 896] bf16
    rmat_d = nc.inline_tensor(rmat_np, "rmatc")
    ident_d = nc.inline_tensor(ident_np, "identc")
    onesc_d = nc.inline_tensor(onesc_np, "onescc")
    onesr_d = nc.inline_tensor(onesr_np.astype(np.float32), "onesrc")  # f32r use

    xstage_d = nc.dram_tensor("xstage", [D, SPC], BF16)     # internal
    xag_d = nc.dram_tensor("xag", [N_CORES * D, SPC], BF16,
                           addr_space="Shared")             # gathered x
    # per-batch partials + reduce-scatter outputs, so batch 0's RS overlaps
    # batch 1's compute
    pout_bd = [nc.dram_tensor(f"pout{b}", [S, D], BF16) for b in range(B)]
    rsout_bd = [nc.dram_tensor(f"rsout{b}", [S // N_CORES, D], BF16)
                for b in range(B)]

    blob_r = blob_d[:].rearrange("(g t p) c -> g p t c", p=128, t=4)  # [4,...]
    xstage_r = xstage_d[:].rearrange("(t p) s -> p t s", p=128)       # 16 t
    xag_r = xag_d[:].rearrange("(n t p) s -> n p t s", p=128, t=4)    # 32 n
    pout_br = [t[:].rearrange("(t p) d -> t p d", p=128) for t in pout_bd]
    rsout_br = [t[:].rearrange("(t p) d -> t p d", p=128) for t in rsout_bd]
    outb_r = outb_d[:].rearrange("(t p) d -> t p d", p=128)           # 4 t

    with tile.TileContext(nc) as tc:
        with (
            nc.allow_low_precision(reason="bf16 compute is intended"),
            tc.tile_pool(name="const", bufs=1) as constp,
            tc.tile_pool(name="xt", bufs=2) as xtp,
            tc.tile_pool(name="qkv", bufs=1) as qkvp,
            tc.tile_pool(name="vh", bufs=2) as vhp,
            tc.tile_pool(name="rope", bufs=2) as ropep,
            tc.tile_pool(name="attn", bufs=5) as attnp,
            tc.tile_pool(name="small", bufs=2) as smallp,
            tc.tile_pool(name="osb", bufs=1) as osbp,
            tc.tile_pool(name="outev", bufs=3) as outevp,
            tc.tile_pool(name="pacc", bufs=6, space="PSUM") as paccp,
            tc.tile_pool(name="pav", bufs=1, space="PSUM") as pavp,
            tc.tile_pool(name="psum1", bufs=1, space="PSUM") as psum1p,
        ):
            # ---- stage x-shard to internal dram, then AllGather ----
            xst = constp.tile([128, 16, SPC], BF16, name="xst")
            nc.sync.dma_start(
                xst[:], blob_d[:].rearrange("(t p) c -> p t c", p=128)
                [:, :, XC:XC + SPC])
            nc.sync.dma_start(xstage_r, xst[:])
            nc.gpsimd.collective_compute(
                "AllGather", mybir.AluOpType.bypass, replica_groups=groups,
                ins=[xstage_d[:].opt()], outs=[xag_d[:].opt()])

            # ---- constants to SBUF ----
            wall_g = []
            for g in range(4):
                wt = constp.tile([128, 4, 1024], BF16, name=f"w_g{g}")
                nc.scalar.dma_start(wt[:], blob_r[g][:, :, WQC:BLOBC])
                wall_g.append(wt)
            cos_sb = constp.tile([128, S], BF16)
            sin_sb = constp.tile([128, S], BF16)
            nc.scalar.dma_start(cos_sb[0:64, :], cos_d[:])
            nc.scalar.dma_start(cos_sb[64:128, :], cos_d[:])
            nc.scalar.dma_start(sin_sb[0:64, :], sin_d[:])
            nc.scalar.dma_start(sin_sb[64:128, :], sin_d[:])
            mask_sb = constp.tile([128, 896], BF16)
            nc.scalar.dma_start(mask_sb[:], mask_d[:])
            rmat_sb = constp.tile([128, 128], BF16)
            ident_sb = constp.tile([128, 128], BF16)
            onesc_sb = constp.tile([128, 1], BF16)
            onesr_sb = constp.tile([1, 128], F32R)
            nc.scalar.dma_start(rmat_sb[:], rmat_d[:])
            nc.scalar.dma_start(ident_sb[:], ident_d[:])
            nc.scalar.dma_start(onesc_sb[:], onesc_d[:])
            nc.gpsimd.dma_start(onesr_sb[:], onesr_d[:])

            # ---- transpose Wo slice [d, f] -> wo_sb [f, 2, d] ----
            wo_sb = constp.tile([128, 2, D], BF16, name="wo_sb")
            for g in range(4):
                for i in range(4):
                    for f2 in range(2):
                        ps_tp = paccp.tile([128, 128], BF16, tag="pacc")
                        nc.tensor.transpose(
                            ps_tp[:],
                            wall_g[g][:, i, 768 + 128 * f2:768 + 128 * (f2 + 1)],
                            ident_sb[:])
                        nc.scalar.copy(wo_sb[:, f2, 512 * g + 128 * i:
                                             512 * g + 128 * i + 128], ps_tp[:])

            for b in range(B):
                o_sb = osbp.tile([128, HPC, S], BF16, tag="o_sb")
                qTs, kTs, vTs = [], [], []
                for h in range(HPC):
                    qTs.append(qkvp.tile([128, S], BF16, name=f"qT{h}", tag=f"qT{h}"))
                    kTs.append(qkvp.tile([128, S], BF16, name=f"kT{h}", tag=f"kT{h}"))
                    vTs.append(qkvp.tile([128, S], BF16, name=f"vT{h}", tag=f"vT{h}"))
                # ---- projections: both heads share each xag tile ----
                for sc in range(NSC):
                    ss = slice(512 * sc, 512 * sc + 512)
                    acc = [paccp.tile([128, 512], F32, name=f"pa{j}", tag="pacc")
                           for j in range(6)]
                    for g in range(4):
                        xt = xtp.tile([128, 4, 512], BF16, tag="xt")
                        eng = nc.sync if g % 2 == 0 else nc.gpsimd
                        eng.dma_start(xt[:], xag_r[(b * NSC + sc) * 4 + g])
                        for i in range(4):
                            kt = 4 * g + i
                            st, sp = kt == 0, kt == NKT - 1
                            for h in range(HPC):
                                fq = slice(128 * h, 128 * h + 128)
                                fk = slice(256 + 128 * h, 256 + 128 * h + 128)
                                fv = slice(512 + 128 * h, 512 + 128 * h + 128)
                                nc.tensor.matmul(acc[h][:], wall_g[g][:, i, fq],
                                                 xt[:, i, :], start=st, stop=sp)
                                nc.tensor.matmul(acc[2 + h][:], wall_g[g][:, i, fk],
                                                 xt[:, i, :], start=st, stop=sp)
                                nc.tensor.matmul(acc[4 + h][:], wall_g[g][:, i, fv],
                                                 xt[:, i, :], start=st, stop=sp)
                    for h in range(HPC):
                        nc.scalar.copy(qTs[h][:, ss], acc[h][:])
                        nc.scalar.copy(kTs[h][:, ss], acc[2 + h][:])
                        nc.scalar.copy(vTs[h][:, ss], acc[4 + h][:])
                for h in range(HPC):
                    qT, kT, vT = qTs[h], kTs[h], vTs[h]
                    # ---- RoPE in place on qT, kT ----
                    for t_ in (qT, kT):
                        for sc in range(NSC):
                            ss = slice(512 * sc, 512 * sc + 512)
                            ps_rot = paccp.tile([128, 512], F32, tag="pacc")
                            nc.tensor.matmul(ps_rot[:], rmat_sb[:], t_[:, ss],
                                             start=True, stop=True)
                            t2 = ropep.tile([128, 512], BF16, tag="ropetmp")
                            nc.vector.tensor_mul(t2[:], ps_rot[:], sin_sb[:, ss])
                            t1 = ropep.tile([128, 512], BF16, tag="ropetmp")
                            nc.vector.tensor_mul(t1[:], t_[:, ss], cos_sb[:, ss])
                            nc.vector.tensor_add(t_[:, ss], t1[:], t2[:])
                    # ---- transpose vT -> v_h [j, jt, d] ----
                    v_h = vhp.tile([128, NJT, 128], BF16, tag="v_h")
                    for jt in range(NJT):
                        js = slice(128 * jt, 128 * jt + 128)
                        ps_tp = paccp.tile([128, 128], BF16, tag="pacc")
                        nc.tensor.transpose(ps_tp[:], vT[:, js], ident_sb[:])
                        nc.scalar.copy(v_h[:, jt, :], ps_tp[:])
                    # ---- attention ----
                    for qc in range(NSC):
                        qs = slice(512 * qc, 512 * qc + 512)
                        ps_av = pavp.tile([128, 512], F32, tag="pav")
                        ps_sum = psum1p.tile([1, 512], F32, tag="psum1")
                        njt = 4 * qc + 4
                        for jt in range(njt):
                            js = slice(128 * jt, 128 * jt + 128)
                            ps_sc = paccp.tile([128, 512], F32, tag="pacc")
                            nc.tensor.matmul(ps_sc[:], kT[:, js], qT[:, qs],
                                             start=True, stop=True)
                            at = attnp.tile([128, 512], BF16, tag="at")
                            nc.scalar.activation(at[:], ps_sc[:], Exp,
                                                 scale=SCALE)
                            if jt >= 4 * qc:
                                mi = 384 - 128 * (jt - 4 * qc)
                                nc.vector.tensor_mul(
                                    at[:], at[:], mask_sb[:, mi:mi + 512])
                            st, sp = jt == 0, jt == njt - 1
                            nc.tensor.matmul(ps_sum[:], onesc_sb[:], at[:],
                                             start=st, stop=sp)
                            nc.tensor.matmul(ps_av[:], v_h[:, jt, :], at[:],
                                             start=st, stop=sp)
                        sums_sb = smallp.tile([1, 512], F32, tag="sums")
                        nc.scalar.copy(sums_sb[:], ps_sum[:])
                        recip = smallp.tile([1, 512], F32R, tag="recip")
                        nc.vector.reciprocal(recip[:], sums_sb[:])
                        ps_bc = paccp.tile([128, 512], F32, tag="pacc")
                        nc.tensor.matmul(ps_bc[:], onesr_sb[:], recip[:],
                                         start=True, stop=True)
                        recipT = smallp.tile([128, 512], F32, tag="recipT")
                        nc.scalar.copy(recipT[:], ps_bc[:])
                        nc.vector.tensor_mul(o_sb[:, h, qs], ps_av[:],
                                             recipT[:])
                # ---- partial output projection, position-major ----
                for st in range(S // 128):
                    sl = slice(128 * st, 128 * st + 128)
                    for dc in range(4):
                        dsl = slice(512 * dc, 512 * dc + 512)
                        ps_o = paccp.tile([128, 512], F32, tag="pacc")
                        nc.tensor.matmul(ps_o[:], o_sb[:, 0, sl],
                                         wo_sb[:, 0, dsl],
                                         start=True, stop=False)
                        nc.tensor.matmul(ps_o[:], o_sb[:, 1, sl],
                                         wo_sb[:, 1, dsl],
                                         start=False, stop=True)
                        outt = outevp.tile([128, 512], BF16, tag="outt")
                        nc.vector.tensor_copy(outt[:], ps_o[:])
                        oeng = nc.sync if (st + dc) % 2 == 0 else nc.gpsimd
                        oeng.dma_start(pout_br[b][st][:, dsl], outt[:])
                # ---- ReduceScatter this batch's partials ----
                nc.gpsimd.collective_compute(
                    "ReduceScatter", mybir.AluOpType.add, replica_groups=groups,
                    ins=[pout_bd[b][:].opt()], outs=[rsout_bd[b][:].opt()])

            # ---- ship back bf16 slices: outb = [rs(b0) 256 | rs(b1) 256] ----
            for b in range(B):
                for t_ in range(2):
                    rt = outevp.tile([128, D], BF16, tag="rsbf")
                    nc.sync.dma_start(rt[:], rsout_br[b][t_])
                    nc.sync.dma_start(outb_r[b * 2 + t_], rt[:])

    nc.compile()
    return nc


_NC_CACHE = None


def _get_nc():
    global _NC_CACHE
    if _NC_CACHE is None:
        _NC_CACHE = build_nc()
    return _NC_CACHE


def _make_in_maps(inputs):
    x = np.ascontiguousarray(np.asarray(inputs["x"]), dtype=np.float32)
    Wq = np.asarray(inputs["Wq"], dtype=np.float32)
    Wk = np.asarray(inputs["Wk"], dtype=np.float32)
    Wv = np.asarray(inputs["Wv"], dtype=np.float32)
    Wo = np.asarray(inputs["Wo"], dtype=np.float32)

    xT = x.reshape(BS, D).T                                  # [D, B*S]
    in_maps = []
    for cid in range(N_CORES):
        f0 = cid * FPC
        blob = np.empty((D, BLOBC), dtype=ml_dtypes.bfloat16)
        blob[:, XC:XC + SPC] = xT[:, cid * SPC:(cid + 1) * SPC]
        blob[:, WQC:WQC + FPC] = Wq[f0:f0 + FPC, :].T
        blob[:, WKC:WKC + FPC] = Wk[f0:f0 + FPC, :].T
        blob[:, WVC:WVC + FPC] = Wv[f0:f0 + FPC, :].T
        blob[:, WOC:WOC + FPC] = Wo[:, f0:f0 + FPC]
        in_maps.append(dict(blob=blob))
    return in_maps


def kernel(x, Wq, Wk, Wv, Wo):
    in_maps = _make_in_maps(dict(x=x, Wq=Wq, Wk=Wk, Wv=Wv, Wo=Wo))
    nc = _get_nc()
    res = run_bass_kernel_spmd(nc, in_maps, core_ids=list(range(N_CORES)))
    # core c's outb rows [0:256) are batch-0 positions [256c, 256c+256),
    # rows [256:512) are batch-1 positions [256c, 256c+256)
    SPB = S // N_CORES
    out = np.empty((BS, D), dtype=np.float32)
    for c in range(N_CORES):
        ob = np.asarray(res.results[c]["outb"], dtype=np.float32)
        out[SPB * c:SPB * (c + 1)] = ob[0:SPB]
        out[S + SPB * c:S + SPB * (c + 1)] = ob[SPB:2 * SPB]
    return np.ascontiguousarray(out).reshape(B, S, D)
